# revision 11
# baseline (speedup 1.0000x reference)
import sys, os
sys.path.insert(0, "/opt/trn_rl_repo")
import numpy as np
import ml_dtypes

import concourse.bass as bass
import concourse.bacc as bacc
import concourse.tile as tile
from concourse import mybir
from concourse.masks import make_identity

P = 8
HEADS = 8
HC = 256
CIN = 256
DH = HC // HEADS          # 32
B, H, W = 4, 128, 128
HP, WP = 136, 136         # padded
NH = NW = 17              # windows per side
NWIN_ROW = 17             # windows per strip
SPX = NWIN_ROW * P * P    # 1088 pixels per strip
NSTRIP_TOT = B * NH       # 68 strips
NCORES = 8
NSTRIP_CORE = 9           # max strips per core (cores 4-7 pad with dummy)
SCALE = 1.0 / np.sqrt(DH)

F32 = mybir.dt.float32
BF16 = mybir.dt.bfloat16


def _build_program(n_strips):
    nc = bacc.Bacc(None, target_bir_lowering=False, debug=False)
    xs_e = nc.declare_dram_parameter("xs", [n_strips, 2, 128, SPX], BF16, isOutput=False)
    wqk_e = nc.declare_dram_parameter("wqk", [2, 128, 512], BF16, isOutput=False)
    wv_e = nc.declare_dram_parameter("wv", [2, 128, 256], BF16, isOutput=False)
    wo_e = nc.declare_dram_parameter("wo", [2, 128, 256], BF16, isOutput=False)
    pos_e = nc.declare_dram_parameter("posr", [2, 128, SPX], F32, isOutput=False)
    bk_e = nc.declare_dram_parameter("bk", [2, 128, 1], F32, isOutput=False)
    bv_e = nc.declare_dram_parameter("bv", [2, 128, 1], F32, isOutput=False)
    bo_e = nc.declare_dram_parameter("bo", [2, 128, 1], F32, isOutput=False)
    ys_e = nc.declare_dram_parameter("ys", [n_strips, 2, 128, SPX], F32, isOutput=True)

    PXG = [(0, 512), (512, 512), (1024, 64)]   # pixel groups per strip

    from contextlib import ExitStack
    with tile.TileContext(nc) as tc, ExitStack() as ctx:
        consts = ctx.enter_context(tc.tile_pool(name="consts", bufs=1))
        xpool = ctx.enter_context(tc.tile_pool(name="x", bufs=2))
        qkpool = ctx.enter_context(tc.tile_pool(name="qk", bufs=2))
        vpool = ctx.enter_context(tc.tile_pool(name="v", bufs=2))
        opool = ctx.enter_context(tc.tile_pool(name="o", bufs=2))
        ypool = ctx.enter_context(tc.tile_pool(name="y", bufs=2))
        espool = ctx.enter_context(tc.tile_pool(name="es", bufs=3))
        smallp = ctx.enter_context(tc.tile_pool(name="small", bufs=4))
        ps_big = ctx.enter_context(tc.tile_pool(name="psb", bufs=2, space="PSUM"))
        ps_s = ctx.enter_context(tc.tile_pool(name="pss", bufs=2, space="PSUM"))
        ps_o = ctx.enter_context(tc.tile_pool(name="pso", bufs=2, space="PSUM"))
        ps_tr = ctx.enter_context(tc.tile_pool(name="pstr", bufs=2, space="PSUM"))

        ident = consts.tile([128, 128], BF16)
        make_identity(nc, ident[:])

        wqk = [consts.tile([128, 512], BF16, name=f"wqk{t}") for t in range(2)]
        wv = [consts.tile([128, 256], BF16, name=f"wv{t}") for t in range(2)]
        wo = [consts.tile([128, 256], BF16, name=f"wo{t}") for t in range(2)]
        posr = [consts.tile([128, SPX], F32, name=f"posr{t}") for t in range(2)]
        bk = [consts.tile([128, 1], F32, name=f"bk{t}") for t in range(2)]
        bv = [consts.tile([128, 1], F32, name=f"bv{t}") for t in range(2)]
        bo = [consts.tile([128, 1], F32, name=f"bo{t}") for t in range(2)]
        for t in range(2):
            nc.sync.dma_start(out=wqk[t], in_=wqk_e[t])
            nc.sync.dma_start(out=wv[t], in_=wv_e[t])
            nc.sync.dma_start(out=wo[t], in_=wo_e[t])
            nc.sync.dma_start(out=posr[t], in_=pos_e[t])
            nc.sync.dma_start(out=bk[t], in_=bk_e[t])
            nc.sync.dma_start(out=bv[t], in_=bv_e[t])
            nc.sync.dma_start(out=bo[t], in_=bo_e[t])

        for s in range(n_strips):
            x_sb = [xpool.tile([128, SPX], BF16, tag=f"x{t}", name=f"x_sb{t}") for t in range(2)]
            for t in range(2):
                nc.sync.dma_start(out=x_sb[t], in_=xs_e[s, t])

            # ---- qk projection: out [512 ch] = 4 chunks of 128 ----
            q_sb = [qkpool.tile([128, SPX], BF16, tag=f"q{c}", name=f"q_sb{c}") for c in range(2)]
            k_sb = [qkpool.tile([128, SPX], BF16, tag=f"k{c}", name=f"k_sb{c}") for c in range(2)]
            for c in range(4):      # 0,1 = q chunks; 2,3 = k chunks
                for g0, gn in PXG:
                    pqk = ps_big.tile([128, 512], F32, tag="psb")
                    for t in range(2):
                        nc.tensor.matmul(pqk[:, :gn], wqk[t][:, 128 * c:128 * c + 128],
                                         x_sb[t][:, g0:g0 + gn],
                                         start=(t == 0), stop=(t == 1))
                    if c < 2:
                        nc.vector.tensor_add(q_sb[c][:, g0:g0 + gn], pqk[:, :gn],
                                             posr[c][:, g0:g0 + gn])
                    else:
                        nc.scalar.activation(k_sb[c - 2][:, g0:g0 + gn], pqk[:, :gn],
                                             mybir.ActivationFunctionType.Identity,
                                             bias=bk[c - 2][:])

            # ---- v projection (W-stationary, [vch, pix]) ----
            v_sb = [vpool.tile([128, SPX], BF16, tag=f"v{c}", name=f"v_sb{c}") for c in range(2)]
            for c in range(2):
                for g0, gn in PXG:
                    pv = ps_big.tile([128, 512], F32, tag="psb")
                    for t in range(2):
                        nc.tensor.matmul(pv[:, :gn], wv[t][:, 128 * c:128 * c + 128],
                                         x_sb[t][:, g0:g0 + gn],
                                         start=(t == 0), stop=(t == 1))
                    nc.scalar.activation(v_sb[c][:, g0:g0 + gn], pv[:, :gn],
                                         mybir.ActivationFunctionType.Identity,
                                         bias=bv[c][:])

            # vT0 [64, 17*264]: transposed v, rows 0-63 (+ones); bdv [128, 17*264]:
            # block-diag per head pair, rows 64-127 filled via partition-shift DMA
            vT0 = vpool.tile([64, NWIN_ROW * 264], BF16, tag="vT0")
            nc.vector.memset(vT0[:], 1.0)
            bdv = vpool.tile([128, NWIN_ROW * 264], BF16, tag="bdv")
            nc.vector.memset(bdv[:], 0.0)
            # bdk [128, 17*512]: per window, chunk c pair pr block at
            # 512w + 256c + 128pr; head hh (0..3) at rows 32hh, cols 64*(hh%2)
            bdk = vpool.tile([128, NWIN_ROW * 512], BF16, tag="bdk", bufs=1)
            nc.vector.memset(bdk[:], 0.0)
            for c in range(2):
                for hh in range(4):
                    for g0, gn in PXG:
                        nw = gn // 64
                        w0 = g0 // 64
                        src = k_sb[c][32 * hh:32 * hh + 32, g0:g0 + gn]
                        src = src.rearrange("p (w q) -> p w q", w=nw)
                        off = 256 * c + 128 * (hh // 2) + 64 * (hh % 2)
                        dst = bdk[32 * hh:32 * hh + 32, :].rearrange(
                            "p (w x) -> p w x", x=512)[:, w0:w0 + nw, off:off + 64]
                        nc.gpsimd.tensor_copy(out=dst, in_=src)

            o_sb = opool.tile([64, NWIN_ROW * 256], BF16, tag="osb")
            y_in = [ypool.tile([128, SPX], BF16, tag=f"yin{c}", name=f"y_in{c}") for c in range(2)]

            for w in range(NWIN_ROW):
                for c in range(2):
                    ptr = ps_tr.tile([64, 128], BF16, tag="ptr")
                    nc.tensor.transpose(ptr[:], v_sb[c][:, 64 * w:64 * w + 64], ident[:])
                    dst = vT0[:, 264 * w + 132 * c:264 * w + 132 * (c + 1)]
                    dst = dst.rearrange("p (h d) -> p h d", h=4)[:, :, 0:32]
                    nc.scalar.activation(dst, ptr[:].rearrange("p (h d) -> p h d", h=4),
                                         mybir.ActivationFunctionType.Copy)
            # scatter vT0 into block-diag bdv: even heads -> rows 0-63 at col
            # 66t, odd heads -> rows 64-127 at col 66t+33  (t = h//2)
            v_even = vT0[:].rearrange("p (w h e) -> p w h e", w=NWIN_ROW, h=8)[:, :, 0::2, :]
            v_odd = vT0[:].rearrange("p (w h e) -> p w h e", w=NWIN_ROW, h=8)[:, :, 1::2, :]
            d_even = bdv[0:64, :].rearrange("p (w t f) -> p w t f", w=NWIN_ROW, t=4)[:, :, :, 0:33]
            d_odd = bdv[64:128, :].rearrange("p (w t f) -> p w t f", w=NWIN_ROW, t=4)[:, :, :, 33:66]
            nc.sync.dma_start(out=d_even, in_=v_even)
            nc.sync.dma_start(out=d_odd, in_=v_odd)

            for w in range(NWIN_ROW):
                # S: zero-padded block-diag, K=128 base 0; pair t = 2c+pr
                pss = ps_s.tile([128, 256], F32, tag="pss")
                for c in range(2):
                    for pr in range(2):
                        t = 2 * c + pr
                        nc.tensor.matmul(
                            pss[:, 64 * t:64 * t + 64],
                            bdk[:, 512 * w + 256 * c + 128 * pr:512 * w + 256 * c + 128 * pr + 128],
                            q_sb[c][:, 64 * w:64 * w + 64],
                            start=True, stop=True)
                es = espool.tile([128, 256], BF16, tag="es")
                nc.scalar.activation(es[:], pss[:],
                                     mybir.ActivationFunctionType.Exp, scale=SCALE)

                # O: K=128 with block-diag v; pair t covers heads 2t, 2t+1
                pso = ps_o.tile([64, 264], F32, tag="pso")
                for t in range(4):
                    nc.tensor.matmul(
                        pso[:, 66 * t:66 * t + 66],
                        es[:, 64 * t:64 * t + 64],
                        bdv[:, 264 * w + 66 * t:264 * w + 66 * t + 66],
                        start=True, stop=True)

                rec = smallp.tile([64, 8], F32, tag="rec")
                nc.vector.reciprocal(out=rec[:],
                                     in_=pso[:].rearrange("p (h e) -> p h e", h=8)[:, :, 32:33])
                ow = o_sb[:, 256 * w:256 * (w + 1)].rearrange("p (h d) -> p h d", h=8)
                nc.vector.tensor_tensor(
                    out=ow,
                    in0=pso[:].rearrange("p (h e) -> p h e", h=8)[:, :, 0:32],
                    in1=rec[:].unsqueeze(2).broadcast_to([64, 8, 32]),
                    op=mybir.AluOpType.mult)

                for c in range(2):
                    ptr2 = ps_tr.tile([128, 64], BF16, tag="ptr")
                    nc.tensor.transpose(ptr2[:], o_sb[:, 256 * w + 128 * c:256 * w + 128 * (c + 1)], ident[0:64, 0:64])
                    nc.scalar.activation(y_in[c][:, 64 * w:64 * w + 64], ptr2[:],
                                         mybir.ActivationFunctionType.Copy)

            # ---- out projection ----
            y_out = [ypool.tile([128, SPX], F32, tag=f"yout{c}", name=f"y_out{c}") for c in range(2)]
            for c in range(2):
                for g0, gn in PXG:
                    py = ps_big.tile([128, 512], F32, tag="psb")
                    for t in range(2):
                        nc.tensor.matmul(py[:, :gn], wo[t][:, 128 * c:128 * c + 128],
                                         y_in[t][:, g0:g0 + gn],
                                         start=(t == 0), stop=(t == 1))
                    nc.scalar.activation(y_out[c][:, g0:g0 + gn], py[:, :gn],
                                         mybir.ActivationFunctionType.Identity,
                                         bias=bo[c][:])
                nc.sync.dma_start(out=ys_e[s, c], in_=y_out[c])
    nc.compile()
    return nc


_prog_cache = {}


def _get_program(n_strips):
    if n_strips not in _prog_cache:
        _prog_cache[n_strips] = _build_program(n_strips)
    return _prog_cache[n_strips]


def _host_prep(x, w_qkv, b_qkv, position, w_out, b_out):
    x = np.asarray(x, np.float32)
    w_qkv = np.asarray(w_qkv, np.float32)
    b_qkv = np.asarray(b_qkv, np.float32)
    position = np.asarray(position, np.float32)
    w_out = np.asarray(w_out, np.float32)
    b_out = np.asarray(b_out, np.float32)

    xp = np.zeros((B, CIN, HP, WP), np.float32)
    xp[:, :, 4:4 + H, 4:4 + W] = x
    # strips: (b, wr) -> [256, 17, 8, 8] window-major
    xs = xp.reshape(B, CIN, NH, P, WP)                  # b c wr r col
    xs = xs.reshape(B, CIN, NH, P, NW, P)               # b c wr r w cw
    xs = xs.transpose(0, 2, 1, 4, 3, 5)                 # b wr c w r cw
    xs = np.ascontiguousarray(xs.reshape(NSTRIP_TOT, CIN, SPX))
    xs = xs.reshape(NSTRIP_TOT, 2, 128, SPX).astype(ml_dtypes.bfloat16)

    wqk = w_qkv[:512].T.reshape(2, 128, 512).astype(ml_dtypes.bfloat16)
    wv = w_qkv[512:].T.reshape(2, 128, 256).astype(ml_dtypes.bfloat16)
    wo = w_out.T.reshape(2, 128, 256).astype(ml_dtypes.bfloat16)

    pos_t = position.reshape(HC, 64) + b_qkv[:HC, None]     # [256, 64] (+q bias)
    posr = np.tile(pos_t, (1, NWIN_ROW)).reshape(2, 128, SPX).astype(np.float32)
    bk = b_qkv[HC:2 * HC].reshape(2, 128, 1).astype(np.float32)
    bv = b_qkv[2 * HC:].reshape(2, 128, 1).astype(np.float32)
    bo = b_out.reshape(2, 128, 1).astype(np.float32)
    return xs, wqk, wv, wo, posr, bk, bv, bo


def kernel(x, w_qkv, b_qkv, position, w_out, b_out):
    from concourse.bass_utils import run_bass_kernel_spmd
    xs, wqk, wv, wo, posr, bk, bv, bo = _host_prep(x, w_qkv, b_qkv, position, w_out, b_out)

    strip_ids = [[c + NCORES * j for j in range(NSTRIP_CORE) if c + NCORES * j < NSTRIP_TOT]
                 for c in range(NCORES)]
    in_maps = []
    for c in range(NCORES):
        ids = strip_ids[c]
        pad = ids + [0] * (NSTRIP_CORE - len(ids))
        in_maps.append({
            "xs": np.ascontiguousarray(xs[pad]),
            "wqk": wqk, "wv": wv, "wo": wo, "posr": posr,
            "bk": bk, "bv": bv, "bo": bo,
        })

    nc = _get_program(NSTRIP_CORE)
    res = run_bass_kernel_spmd(nc, in_maps, list(range(NCORES))).results

    yp = np.zeros((B, CIN, HP, WP), np.float32)
    for c in range(NCORES):
        ys = res[c]["ys"].reshape(NSTRIP_CORE, CIN, NWIN_ROW, P, P)
        for j, sid in enumerate(strip_ids[c]):
            b, wr = divmod(sid, NH)
            yp[b, :, 8 * wr:8 * wr + 8, :] = (
                ys[j].transpose(0, 2, 1, 3).reshape(CIN, P, WP))
    return np.ascontiguousarray(yp[:, :, 4:4 + H, 4:4 + W])
